# revision 8
# baseline (speedup 1.0000x reference)
"""Trainium2 Bass kernel for LocalGlobalSelfAttention (v4).

Sharding: 8 cores = 4 batches x 2 sequence-halves (no collectives).
Each core computes, for its (batch b, half h):
  - global attention: queries = its half (SH rows), keys/values = full seq
  - local windowed attention: fully contained in its half
  - output projections (g+l accumulated in PSUM) + residual + layernorm

v4 changes over v3 (566us baseline):
  - exp split across ScalarE (exact, fp8 out) and DVE (Schraudolph: one
    tensor_scalar affine + f32->uint8 convert whose bits, read as fp8e4m3,
    approximate 2^x; softmax renormalization cancels the systematic bias)
  - softmax reciprocal on ScalarE as exp(-ln r) (same activation table set
    as Exp -> no table thrash); paired [65,2,QC] PSUM o tiles so rowsum
    extraction is one op per (hp, chunk)
  - bv folded into the output-projection bias, and that + bo folded into
    the residual input xq host-side; V tiles grouped per head-pair-group so
    the PSUM->SBUF evac is one copy per s-tile
  - out-projection in fp8 DoubleRow over merged o/wo tiles (pair order
    g0..g6,l0..l6,g7,l7 so the last pair is exactly what the final phase
    adds); wo scaled x4 host-side, o scaled /4 via the reciprocal bias to
    dodge fp8 subnormals
  - layernorm via bn_stats + fused tensor_scalar, gamma/beta on GpSimd,
    bf16 output
"""

import numpy as np
import ml_dtypes
from contextlib import ExitStack

BF16 = ml_dtypes.bfloat16
FP8 = ml_dtypes.float8_e4m3

FULL_CFG = dict(S=2048, D=1024, H=16, K=64, NW=8)
N_CORES = 8
LN_EPS = 1e-3

LOG2E = 1.4426950408889634
SCHRAU_B = 55.65          # exp bias for round-to-nearest f32->uint8
OSCALE = 1.0              # optional wo x / o ÷ scale split (fp8 headroom)
# exp instances assigned to DVE when (counter % 8) in this set
EXP_DVE_SLOTS = (2, 5, 7)


def _chunks(total, size):
    return [(o, min(size, total - o)) for o in range(0, total, size)]


def build_nc(cfg=None, n_dev=N_CORES, exp_dve_slots=EXP_DVE_SLOTS,
             kq_evac_scalar_every=0):
    """Build + compile the per-core Bass program (SPMD, same on all cores)."""
    import concourse.bass as bass
    import concourse.tile as tile
    import concourse.mybir as mybir
    from concourse import bacc

    cfg = dict(cfg or FULL_CFG)
    S, D, H, K, NW = cfg["S"], cfg["D"], cfg["H"], cfg["K"], cfg["NW"]
    HK = H * K
    SH = S // 2          # per-core query rows (half the sequence)
    WIN = S // NW        # local attention window
    NWH = SH // WIN      # windows in this core's half
    assert K == 64 and D % 128 == 0 and HK % 128 == 0

    ND = D // 128        # d-tiles
    NHK = HK // 128      # head-pair tiles (2 heads each)
    NST = S // 128       # s-tiles (full seq)
    NQT = SH // 128      # q-tiles (half seq)
    QC = 512             # query chunk for AV / o accumulation
    NQC = SH // QC
    NGRP = 2             # v-projection groups (4 head-pairs each)
    GHP = NHK // NGRP
    NP = 2 * NHK         # merged out-projection tiles (g and l)

    f32 = mybir.dt.float32
    bf16 = mybir.dt.bfloat16
    fp8 = mybir.dt.float8e4
    u8 = mybir.dt.uint8
    DR = mybir.MatmulPerfMode.DoubleRow
    Exp = mybir.ActivationFunctionType.Exp
    Ln = mybir.ActivationFunctionType.Ln
    Copy = mybir.ActivationFunctionType.Copy
    Sqrt = mybir.ActivationFunctionType.Sqrt
    add_op = mybir.AluOpType.add
    mult_op = mybir.AluOpType.mult
    sub_op = mybir.AluOpType.subtract

    # out-projection tile order: g0..g6, l0..l6, g7, l7
    def p_of(st, hp):
        if hp < NHK - 1:
            return hp if st == "g" else (NHK - 1) + hp
        return NP - 2 if st == "g" else NP - 1

    nc = bacc.Bacc("TRN2", target_bir_lowering=False, debug=False,
                   num_devices=n_dev)

    # ---- DRAM parameters -------------------------------------------------
    xT_d = nc.dram_tensor("xT", [D, S], fp8, kind="ExternalInput")
    xq_d = nc.dram_tensor("xq", [SH, D], f32, kind="ExternalInput")
    w_d = {}
    for nm in ("wq_g", "wk_g", "wq_l", "wk_l"):
        w_d[nm] = nc.dram_tensor(nm, [NHK, ND, 128, 128], fp8,
                                 kind="ExternalInput")
    wv_d = {}
    for nm in ("wv_g", "wv_l"):
        wv_d[nm] = nc.dram_tensor(nm, [NGRP, ND, 128, GHP * 128], fp8,
                                  kind="ExternalInput")
    # merged wo, pre-ordered host-side to the p_of layout, scaled x4, fp8
    wo_d = nc.dram_tensor("wo", [NP, 128, D], fp8, kind="ExternalInput")
    bcol_d = {}
    for nm in ("bq_g", "bk_g", "bq_l", "bk_l"):
        bcol_d[nm] = nc.dram_tensor(nm, [NHK, 128], f32, kind="ExternalInput")
    gamma_d = nc.dram_tensor("gamma", [1, D], bf16, kind="ExternalInput")
    beta_d = nc.dram_tensor("beta", [1, D], bf16, kind="ExternalInput")
    out_d = nc.dram_tensor("out", [SH, D], bf16, kind="ExternalOutput")

    PS = bass.MemorySpace.PSUM

    with tile.TileContext(nc) as tc, ExitStack() as ctx:
        # ---- constants (live whole kernel) -------------------------------
        cpool = ctx.enter_context(tc.tile_pool(name="consts", bufs=1))
        eps_col = cpool.tile([128, 1], f32, tag="eps", name="eps")
        nc.vector.memset(eps_col[:], float(LN_EPS))
        bcol_sb = {}
        for nm, d in bcol_d.items():
            cols = []
            for j in range(NHK):
                t = cpool.tile([128, 1], f32, tag=f"{nm}{j}", name=f"{nm}{j}")
                nc.sync.dma_start(t[:], d[j, :].rearrange("(a b) -> a b", b=1))
                cols.append(t)
            bcol_sb[nm] = cols

        # x^T resident in SBUF for all projections
        xpool = ctx.enter_context(tc.tile_pool(name="xin", bufs=1))
        xTp = [xpool.tile([128, 2, S], fp8, tag=f"xt{dp}", name=f"xt{dp}")
               for dp in range(ND // 2)]
        for dp in range(ND // 2):
            for j in range(2):
                nc.sync.dma_start(
                    xTp[dp][:, j, :],
                    xT_d[(2 * dp + j) * 128:(2 * dp + j + 1) * 128, :])

        # merged o accumulator (fp8, /OSCALE) for the out-projection
        opool = ctx.enter_context(tc.tile_pool(name="oacc", bufs=1))
        o_all = opool.tile([128, NP, SH], fp8, tag="oall", name="oall")

        # merged wo (fp8, xOSCALE), prefetched during the pipeline
        wop = ctx.enter_context(tc.tile_pool(name="wo", bufs=1))
        wo_all = {}

        def load_wo():
            wo_all["t"] = wop.tile([128, NP, D], fp8, tag="woall",
                                   name="woall")
            for p in range(NP):
                nc.sync.dma_start(wo_all["t"][:, p, :], wo_d[p])

        # final-phase tiles that must coexist with the attention pools
        fin = ctx.enter_context(tc.tile_pool(name="fin", bufs=1))
        gamma_bc = fin.tile([128, D], bf16, tag="gamma", name="gamma",
                            bufs=1)
        nc.sync.dma_start(gamma_bc[:], gamma_d[:].partition_broadcast(128))
        beta_bc = fin.tile([128, D], bf16, tag="beta", name="beta", bufs=1)
        nc.sync.dma_start(beta_bc[:], beta_d[:].partition_broadcast(128))
        y_tiles = {}

        # ---- per-head-pair pools (double buffered across hp) -------------
        hp_ctx = ExitStack()
        kqv = hp_ctx.enter_context(tc.tile_pool(name="kqv", bufs=2))
        wts = hp_ctx.enter_context(tc.tile_pool(name="wts", bufs=2))
        vxp = hp_ctx.enter_context(tc.tile_pool(name="vxp", bufs=2))
        ppsum = hp_ctx.enter_context(
            tc.tile_pool(name="ppsum", bufs=2, space=PS))
        scp = hp_ctx.enter_context(tc.tile_pool(name="scp", bufs=2, space=PS))
        ovp = hp_ctx.enter_context(tc.tile_pool(name="ovp", bufs=1, space=PS))
        exp_p = hp_ctx.enter_context(tc.tile_pool(name="exp", bufs=3))
        nop = hp_ctx.enter_context(tc.tile_pool(name="norm", bufs=1))

        exp_ctr = [0]

        def emit_exp(dst_fp8, src_psum):
            """One [128, 1024] exp: ScalarE exact or DVE Schraudolph."""
            i = exp_ctr[0]
            exp_ctr[0] += 1
            if (i % 8) in exp_dve_slots:
                nc.vector.tensor_scalar(
                    dst_fp8.bitcast(u8), src_psum, LOG2E, SCHRAU_B,
                    mult_op, add_op)
            else:
                nc.scalar.activation(dst_fp8, src_psum, Exp, scale=0.125)

        evac_ctr = [0]

        def emit_kq_evac(dst, src_psum, bias):
            i = evac_ctr[0]
            evac_ctr[0] += 1
            if kq_evac_scalar_every and i % kq_evac_scalar_every == 0:
                nc.scalar.activation(dst, src_psum, Copy, bias=bias)
            else:
                nc.vector.tensor_scalar(dst, src_psum, bias, None, add_op)

        def load_kq_weights(hp):
            out = {}
            for nm in ("wk_g", "wq_g", "wk_l", "wq_l"):
                t = wts.tile([128, ND * 128], fp8, tag=nm, name=nm)
                for d in range(ND):
                    nc.sync.dma_start(t[:, d * 128:(d + 1) * 128],
                                      w_d[nm][hp, d])
                out[nm] = t
            return out

        def load_v_weights(grp):
            out = {}
            for nm in ("wv_g", "wv_l"):
                t = wts.tile([128, ND, GHP * 128], fp8, tag=f"{nm}4",
                             name=f"{nm}4", bufs=1)
                for d in range(ND):
                    nc.sync.dma_start(t[:, d, :], wv_d[nm][grp, d])
                out[nm] = t
            return out

        def vproj_gen(grp, wv, dst):
            """V projections for head-pair group grp (4 head-pairs).
            dst['vg'/'vl'] maps hp -> grouped tile [128, 2sub, 2j, GHP, 80]
            (all GHP head-pairs share one tile per (key, t-pair))."""
            for nm, n_t, key in (("wv_g", NST, "vg"), ("wv_l", NQT, "vl")):
                tiles = []
                for t in range(n_t):
                    pt = ppsum.tile([128, 512], f32, tag="pp", name="pp")
                    for dp in range(ND // 2):
                        nc.tensor.matmul(
                            pt[:, 0:GHP * 128],
                            xTp[dp][:, :, t * 128:(t + 1) * 128],
                            wv[nm][:].rearrange(
                                "p (dp j) c -> p dp j c", j=2)[:, dp],
                            start=(dp == 0), stop=(dp == ND // 2 - 1),
                            perf_mode=DR)
                    if t % 2 == 0:
                        vt = vxp.tile([128, 2, 2, GHP, 80], fp8,
                                      tag=f"{key}{t // 2}",
                                      name=f"{key}{t // 2}")
                        tiles.append(vt)
                        dst[key][grp * (n_t // 2) + t // 2] = vt
                        nc.vector.memset(vt[:, :, :, :, 64:65], 1.0)
                    vt = tiles[t // 2]
                    # one evac per s-tile: [128, (sub, ghp, 64)] <- pt
                    nc.vector.tensor_copy(
                        vt[:, :, t % 2, :, 0:64],
                        pt[:, 0:GHP * 128].rearrange(
                            "p (i s k) -> p s i k", s=2, k=64))
                    if t % 2 == 1:
                        yield

        def proj_gen(hp, w, dst):
            """kq projections for head-pair hp: out[hkp, s] = (x@w)^T + b."""
            for nm, s_len, key in (("wk_g", S, "kg"), ("wq_g", SH, "qg"),
                                   ("wk_l", SH, "kl"), ("wq_l", SH, "ql")):
                ot = kqv.tile([128, s_len], bf16, tag=key, name=key)
                dst[key] = ot
                bias = bcol_sb["b" + nm[1:]][hp]
                for so, sl in _chunks(s_len, 512):
                    pt = ppsum.tile([128, 512], f32, tag="pp", name="pp")
                    for dp in range(ND // 2):
                        nc.tensor.matmul(
                            pt[:, 0:sl],
                            w[nm][:].rearrange(
                                "p (dp j c) -> p dp j c", j=2, c=128)[:, dp],
                            xTp[dp][:, :, so:so + sl],
                            start=(dp == 0), stop=(dp == ND // 2 - 1),
                            perf_mode=DR)
                    emit_kq_evac(ot[:, so:so + sl], pt[:, 0:sl], bias)
                    yield

        def normalize(opair, qo, hp, st, tagix):
            """softmax-normalize the o pair [65, 2, QC] into o_all; rowsum
            reciprocal (and the /OSCALE) fused on ScalarE as exp(-ln r)."""
            lnr = nop.tile([1, 2, QC], f32, tag=f"lr{tagix}",
                           name=f"lr{tagix}")
            nc.scalar.activation(lnr[:], opair[64:65, :, :], Ln)
            rinv = nop.tile([1, 2, QC], f32, tag=f"ri{tagix}",
                            name=f"ri{tagix}")
            nc.scalar.activation(rinv[:], lnr[:], Exp, scale=-1.0,
                                 bias=-float(np.log(OSCALE)))
            p = p_of(st, hp)
            for sub in range(2):
                rb = nop.tile([64, QC], f32, tag=f"rb{sub}{tagix}",
                              name=f"rb{sub}{tagix}")
                nc.gpsimd.partition_broadcast(rb[:], rinv[0:1, sub, :])
                nc.vector.tensor_tensor(
                    o_all[sub * 64:sub * 64 + 64, p, qo:qo + QC],
                    opair[0:64, sub, :], rb[:], mult_op)

        def attn_gen(hp, src):
            """Attention (global + local) for head-pair hp."""
            kg, qg = src["kg"], src["qg"]
            kl, ql_ = src["kl"], src["ql"]
            # ---- global: q-chunks outer, s-tile-pairs inner --------------
            for qc in range(NQC):
                qo = qc * QC
                opair = ovp.tile([65, 2, QC], f32, tag="o", name="o")
                for tp in range(NST // 2):
                    ex = exp_p.tile([128, 2, 2, QC], fp8, tag="ex", name="ex")
                    for j in range(2):
                        t = 2 * tp + j
                        sc = scp.tile([128, 2, QC], f32, tag="sc", name="sc")
                        for sub in range(2):
                            po = sub * 64
                            nc.tensor.matmul(
                                sc[:, sub, :],
                                kg[po:po + 64, t * 128:(t + 1) * 128],
                                qg[po:po + 64, qo:qo + QC],
                                start=True, stop=True)
                        emit_exp(ex[:, j], sc[:])
                    for sub in range(2):
                        vt = vkeys["vg"][(hp // GHP) * (NST // 2) + tp]
                        nc.tensor.matmul(
                            opair[:, sub, :],
                            vt[:, sub, :, hp % GHP, 0:65],
                            ex[:, :, sub, :],
                            start=(tp == 0), stop=(tp == NST // 2 - 1),
                            perf_mode=DR)
                    yield
                normalize(opair, qo, hp, "g", qc)
                yield
            # ---- local: q-chunk = window pair, DoubleRow over ss ---------
            for wp in range(NWH // 2):
                qo = wp * QC
                opair = ovp.tile([65, 2, QC], f32, tag="o", name="o")
                ex = exp_p.tile([128, 2, 2, QC], fp8, tag="ex", name="ex")
                for ss in range(2):
                    sc = scp.tile([128, 2, QC], f32, tag="sc", name="sc")
                    for sub in range(2):
                        po = sub * 64
                        for wi in range(2):
                            w = 2 * wp + wi
                            st_ = 2 * w + ss
                            nc.tensor.matmul(
                                sc[:, sub, wi * 256:wi * 256 + 256],
                                kl[po:po + 64, st_ * 128:(st_ + 1) * 128],
                                ql_[po:po + 64,
                                    qo + wi * 256:qo + wi * 256 + 256],
                                start=(wi == 0), stop=(wi == 1))
                    emit_exp(ex[:, ss], sc[:])
                for sub in range(2):
                    for wi in range(2):
                        w = 2 * wp + wi
                        vt = vkeys["vl"][(hp // GHP) * (NQT // 2) + w]
                        nc.tensor.matmul(
                            opair[:, sub, wi * 256:wi * 256 + 256],
                            vt[:, sub, :, hp % GHP, 0:65],
                            ex[:, :, sub, wi * 256:wi * 256 + 256],
                            start=(wi == 0), stop=(wi == 1),
                            perf_mode=DR)
                yield
                normalize(opair, qo, hp, "l", 2 + wp)
                yield

        def outproj_partial_gen():
            """Out-projection partial sums (pairs 0..NP//2-2) + residual,
            overlapped with the last attention; leaves y = x + bo + partial.
            (bo_eff is folded into xq host-side.)"""
            wo_t = wo_all["t"]
            for qt in range(NQT):
                xq_t = fin.tile([128, D], f32, tag="xq", name="xq", bufs=2)
                nc.sync.dma_start(xq_t[:], xq_d[qt * 128:(qt + 1) * 128, :])
                y = fin.tile([128, D], bf16, tag="y", name="y", bufs=NQT)
                y_tiles[qt] = y
                for do, dl in _chunks(D, 512):
                    pt = ppsum.tile([128, 512], f32, tag="pp", name="pp")
                    for k in range(NP // 2 - 1):
                        nc.tensor.matmul(
                            pt[:, 0:dl],
                            o_all[:, 2 * k:2 * k + 2,
                                  qt * 128:(qt + 1) * 128],
                            wo_t[:, 2 * k:2 * k + 2, do:do + dl],
                            start=(k == 0), stop=(k == NP // 2 - 2),
                            perf_mode=DR)
                    nc.vector.tensor_tensor(
                        y[:, do:do + dl], pt[:, 0:dl],
                        xq_t[:, do:do + dl], add_op)
                    yield

        # ---- software-pipelined main loop over head-pairs ----------------
        import itertools
        kq_w = load_kq_weights(0)
        v_w = load_v_weights(0)
        src = {}
        prev_src = None
        vkeys = {"vg": {}, "vl": {}}
        for hp in range(NHK):
            if hp == 0:
                pg = itertools.chain(proj_gen(hp, kq_w, src),
                                     vproj_gen(0, v_w, vkeys))
            elif hp % GHP == 0:
                pg = itertools.chain(vproj_gen(hp // GHP, v_w, vkeys),
                                     proj_gen(hp, kq_w, src))
            else:
                pg = proj_gen(hp, kq_w, src)
            ag = attn_gen(hp - 1, prev_src) if prev_src is not None else None
            if hp == 2:
                load_wo()
            if hp + 1 < NHK:
                kq_next = load_kq_weights(hp + 1)
            if hp % GHP == 1 and hp + GHP < NHK + 1:
                v_next = load_v_weights((hp + GHP) // GHP)
            # interleave: 2 attention steps per projection step
            done_p, done_a = False, ag is None
            while not (done_p and done_a):
                if not done_a:
                    done_a = next(ag, "END") == "END"
                if not done_p:
                    done_p = next(pg, "END") == "END"
                if not done_a:
                    done_a = next(ag, "END") == "END"
            if hp + 1 < NHK:
                kq_w = kq_next
            if hp % GHP == 1 and hp + GHP < NHK + 1:
                v_w = v_next
            prev_src, src = src, {}
        ag = attn_gen(NHK - 1, prev_src)
        pg = outproj_partial_gen()
        done_p = done_a = False
        while not (done_p and done_a):
            if not done_a:
                done_a = next(ag, "END") == "END"
            if not done_p:
                done_p = next(pg, "END") == "END"
            if not done_a:
                done_a = next(ag, "END") == "END"
        hp_ctx.close()

        # ---- finish: last pair (g7, l7), then layernorm ------------------
        wo_t = wo_all["t"]
        with tc.tile_pool(name="ypsum", bufs=2, space=PS) as ypp, \
             tc.tile_pool(name="ln", bufs=2) as lnp:
            for qt in range(NQT):
                y = y_tiles[qt]
                for do, dl in _chunks(D, 512):
                    ps_y = ypp.tile([128, 512], f32, tag="py", name="py")
                    nc.tensor.matmul(
                        ps_y[:, 0:dl],
                        o_all[:, NP - 2:NP, qt * 128:(qt + 1) * 128],
                        wo_t[:, NP - 2:NP, do:do + dl],
                        start=True, stop=True, perf_mode=DR)
                    nc.vector.tensor_tensor(y[:, do:do + dl], y[:, do:do + dl],
                                            ps_y[:, 0:dl], add_op)
                # layernorm: bn_stats/aggr for mean+var in two passes
                st6 = lnp.tile([128, 2, 6], f32, tag="st6", name="st6")
                nc.vector.bn_stats(st6[:, 0, :], y[:, 0:512])
                nc.vector.bn_stats(st6[:, 1, :], y[:, 512:1024])
                mv = lnp.tile([128, 2], f32, tag="mv", name="mv")
                nc.vector.bn_aggr(mv[:], st6[:])
                sd = lnp.tile([128, 1], f32, tag="sd", name="sd")
                nc.scalar.activation(sd[:], mv[:, 1:2], Sqrt,
                                     bias=eps_col[:])
                rstd = lnp.tile([128, 1], f32, tag="rstd", name="rstd")
                nc.vector.reciprocal(rstd[:], sd[:])
                bco = lnp.tile([128, 1], f32, tag="bco", name="bco")
                nc.vector.tensor_tensor(bco[:], mv[:, 0:1], rstd[:], mult_op)
                nc.vector.tensor_scalar_mul(bco[:], bco[:], -1.0)
                t1 = lnp.tile([128, D], bf16, tag="t1", name="t1")
                nc.vector.tensor_scalar(t1[:], y[:], rstd[:], bco[:],
                                        mult_op, add_op)
                t2 = lnp.tile([128, D], bf16, tag="t2", name="t2")
                nc.vector.tensor_tensor(t2[:], t1[:], gamma_bc[:], mult_op)
                ot = lnp.tile([128, D], bf16, tag="ot", name="ot")
                nc.vector.tensor_tensor(ot[:], t2[:], beta_bc[:], add_op)
                nc.sync.dma_start(out_d[qt * 128:(qt + 1) * 128, :], ot[:])

    nc.compile()
    return nc


def make_in_maps(inputs, cfg=None):
    """Build per-core input maps from the full (unsharded) problem inputs."""
    cfg = dict(cfg or FULL_CFG)
    S, D, H, K = cfg["S"], cfg["D"], cfg["H"], cfg["K"]
    HK = H * K
    SH = S // 2
    NHK = HK // 128
    ND = D // 128
    NGRP = 2
    GW = HK // NGRP
    NP = 2 * NHK

    def np32(a):
        return np.asarray(a, dtype=np.float32)

    shared = {}
    for nm, key in (("wq_g", "gWq"), ("wk_g", "gWk"),
                    ("wq_l", "lWq"), ("wk_l", "lWk")):
        w = np32(inputs[key]).reshape(D, HK)
        shared[nm] = np.ascontiguousarray(
            w.reshape(ND, 128, NHK, 128).transpose(2, 0, 1, 3)).astype(FP8)
    for nm, key in (("wv_g", "gWv"), ("wv_l", "lWv")):
        w = np32(inputs[key]).reshape(D, HK)
        shared[nm] = np.ascontiguousarray(
            w.reshape(ND, 128, NGRP, GW).transpose(2, 0, 1, 3)).astype(FP8)
    # merged wo in p_of order: g0..g6, l0..l6, g7, l7; scaled by OSCALE
    wo_g = np32(inputs["gWo"]).reshape(HK, D)
    wo_l = np32(inputs["lWo"]).reshape(HK, D)
    wo = np.empty((NP, 128, D), np.float32)
    for hp in range(NHK):
        pg = hp if hp < NHK - 1 else NP - 2
        pl = (NHK - 1) + hp if hp < NHK - 1 else NP - 1
        wo[pg] = wo_g[hp * 128:(hp + 1) * 128]
        wo[pl] = wo_l[hp * 128:(hp + 1) * 128]
    shared["wo"] = (wo * OSCALE).astype(FP8)
    for nm, key in (("bq_g", "gbq"), ("bk_g", "gbk"),
                    ("bq_l", "lbq"), ("bk_l", "lbk")):
        shared[nm] = np.ascontiguousarray(np32(inputs[key]).reshape(NHK, 128))
    # bv folds into the out-projection bias; that + bo fold into xq
    bo_eff = (np32(inputs["gbo"]) + np32(inputs["lbo"]) +
              np32(inputs["gbv"]).reshape(HK) @ wo_g +
              np32(inputs["lbv"]).reshape(HK) @ wo_l)
    shared["gamma"] = np32(inputs["gamma"]).reshape(1, D).astype(BF16)
    shared["beta"] = np32(inputs["beta"]).reshape(1, D).astype(BF16)

    x = np32(inputs["x"])
    in_maps = []
    for c in range(N_CORES):
        b, half = divmod(c, 2)
        xb = x[b]
        # own half first (queries/local), other half second; global attention
        # is invariant to key/value column order
        xperm = np.concatenate([xb[half * SH:(half + 1) * SH],
                                xb[(1 - half) * SH:(2 - half) * SH]], axis=0)
        m = dict(shared)
        m["xT"] = np.ascontiguousarray(xperm.T).astype(FP8)
        m["xq"] = np.ascontiguousarray(xperm[0:SH] + bo_eff[None, :])
        in_maps.append(m)
    return in_maps


def assemble_out(results, cfg=None):
    cfg = dict(cfg or FULL_CFG)
    S, D = cfg["S"], cfg["D"]
    SH = S // 2
    B = N_CORES // 2
    out = np.empty((B, S, D), np.float32)
    for c in range(N_CORES):
        b, half = divmod(c, 2)
        out[b, half * SH:(half + 1) * SH] = np.asarray(
            results[c]["out"]).astype(np.float32)
    return out


_NC_CACHE = {}


def kernel(**inputs):
    from concourse.bass_utils import run_bass_kernel_spmd
    if "nc" not in _NC_CACHE:
        _NC_CACHE["nc"] = build_nc()
    nc = _NC_CACHE["nc"]
    in_maps = make_in_maps(inputs)
    res = run_bass_kernel_spmd(nc, in_maps, list(range(N_CORES)))
    return assemble_out(res.results)


# revision 11
# speedup vs baseline: 1.1010x; 1.1010x over previous
"""Trainium2 Bass kernel for LocalGlobalSelfAttention (v4).

Sharding: 8 cores = 4 batches x 2 sequence-halves (no collectives).
Each core computes, for its (batch b, half h):
  - global attention: queries = its half (SH rows), keys/values = full seq
  - local windowed attention: fully contained in its half
  - output projections (g+l accumulated in PSUM) + residual + layernorm

v4 changes over v3 (566us baseline):
  - exp split across ScalarE (exact, fp8 out) and DVE (Schraudolph: one
    tensor_scalar affine + f32->uint8 convert whose bits, read as fp8e4m3,
    approximate 2^x; softmax renormalization cancels the systematic bias)
  - softmax reciprocal on ScalarE as exp(-ln r) (same activation table set
    as Exp -> no table thrash); paired [65,2,QC] PSUM o tiles so rowsum
    extraction is one op per (hp, chunk)
  - bv folded into the output-projection bias, and that + bo folded into
    the residual input xq host-side; V tiles grouped per head-pair-group so
    the PSUM->SBUF evac is one copy per s-tile
  - out-projection in fp8 DoubleRow over merged o/wo tiles (pair order
    g0..g6,l0..l6,g7,l7 so the last pair is exactly what the final phase
    adds); wo scaled x4 host-side, o scaled /4 via the reciprocal bias to
    dodge fp8 subnormals
  - layernorm via bn_stats + fused tensor_scalar, gamma/beta on GpSimd,
    bf16 output
"""

import numpy as np
import ml_dtypes
from contextlib import ExitStack

BF16 = ml_dtypes.bfloat16
FP8 = ml_dtypes.float8_e4m3

FULL_CFG = dict(S=2048, D=1024, H=16, K=64, NW=8)
N_CORES = 8
LN_EPS = 1e-3

LOG2E = 1.4426950408889634
SCHRAU_B = 55.65          # exp bias for round-to-nearest f32->uint8
OSCALE = 1.0              # optional wo x / o ÷ scale split (fp8 headroom)
# exp instances assigned to DVE when (counter % 8) in this set
EXP_DVE_SLOTS = (2, 6)


def _chunks(total, size):
    return [(o, min(size, total - o)) for o in range(0, total, size)]


def build_nc(cfg=None, n_dev=N_CORES, exp_dve_slots=EXP_DVE_SLOTS,
             kq_evac_scalar_every=0):
    """Build + compile the per-core Bass program (SPMD, same on all cores)."""
    import concourse.bass as bass
    import concourse.tile as tile
    import concourse.mybir as mybir
    from concourse import bacc

    cfg = dict(cfg or FULL_CFG)
    S, D, H, K, NW = cfg["S"], cfg["D"], cfg["H"], cfg["K"], cfg["NW"]
    HK = H * K
    SH = S // 2          # per-core query rows (half the sequence)
    WIN = S // NW        # local attention window
    NWH = SH // WIN      # windows in this core's half
    assert K == 64 and D % 128 == 0 and HK % 128 == 0

    ND = D // 128        # d-tiles
    NHK = HK // 128      # head-pair tiles (2 heads each)
    NST = S // 128       # s-tiles (full seq)
    NQT = SH // 128      # q-tiles (half seq)
    QC = 512             # query chunk for AV / o accumulation
    NQC = SH // QC
    NGRP = 2             # v-projection groups (4 head-pairs each)
    GHP = NHK // NGRP
    NP = 2 * NHK         # merged out-projection tiles (g and l)

    f32 = mybir.dt.float32
    bf16 = mybir.dt.bfloat16
    fp8 = mybir.dt.float8e4
    u8 = mybir.dt.uint8
    u32 = mybir.dt.uint32
    DR = mybir.MatmulPerfMode.DoubleRow
    Exp = mybir.ActivationFunctionType.Exp
    Copy = mybir.ActivationFunctionType.Copy
    Sqrt = mybir.ActivationFunctionType.Sqrt
    add_op = mybir.AluOpType.add
    mult_op = mybir.AluOpType.mult
    sub_op = mybir.AluOpType.subtract

    # out-projection tile order: g0..g6, l0..l6, g7, l7
    def p_of(st, hp):
        if hp < NHK - 1:
            return hp if st == "g" else (NHK - 1) + hp
        return NP - 2 if st == "g" else NP - 1

    nc = bacc.Bacc("TRN2", target_bir_lowering=False, debug=False,
                   num_devices=n_dev)

    # ---- DRAM parameters -------------------------------------------------
    xT_d = nc.dram_tensor("xT", [D, S], fp8, kind="ExternalInput")
    xq_d = nc.dram_tensor("xq", [SH, D], f32, kind="ExternalInput")
    w_d = {}
    for nm in ("wq_g", "wk_g", "wq_l", "wk_l"):
        w_d[nm] = nc.dram_tensor(nm, [NHK, ND, 128, 128], fp8,
                                 kind="ExternalInput")
    wv_d = {}
    for nm in ("wv_g", "wv_l"):
        wv_d[nm] = nc.dram_tensor(nm, [NGRP, ND, 128, GHP * 128], fp8,
                                  kind="ExternalInput")
    # merged wo, pre-ordered host-side to the p_of layout, scaled x4, fp8
    wo_d = nc.dram_tensor("wo", [NP, 128, D], fp8, kind="ExternalInput")
    bcol_d = {}
    for nm in ("bq_g", "bk_g", "bq_l", "bk_l"):
        bcol_d[nm] = nc.dram_tensor(nm, [NHK, 128], f32, kind="ExternalInput")
    gamma_d = nc.dram_tensor("gamma", [1, D], bf16, kind="ExternalInput")
    beta_d = nc.dram_tensor("beta", [1, D], bf16, kind="ExternalInput")
    out_d = nc.dram_tensor("out", [SH, D], bf16, kind="ExternalOutput")

    PS = bass.MemorySpace.PSUM

    with tile.TileContext(nc) as tc, ExitStack() as ctx:
        # ---- constants (live whole kernel) -------------------------------
        cpool = ctx.enter_context(tc.tile_pool(name="consts", bufs=1))
        eps_col = cpool.tile([128, 1], f32, tag="eps", name="eps")
        nc.vector.memset(eps_col[:], float(LN_EPS))
        bcol_sb = {}
        for nm, d in bcol_d.items():
            cols = []
            for j in range(NHK):
                t = cpool.tile([128, 1], f32, tag=f"{nm}{j}", name=f"{nm}{j}")
                nc.sync.dma_start(t[:], d[j, :].rearrange("(a b) -> a b", b=1))
                cols.append(t)
            bcol_sb[nm] = cols

        # x^T resident in SBUF for all projections
        xpool = ctx.enter_context(tc.tile_pool(name="xin", bufs=1))
        xTp = [xpool.tile([128, 2, S], fp8, tag=f"xt{dp}", name=f"xt{dp}")
               for dp in range(ND // 2)]
        for dp in range(ND // 2):
            for j in range(2):
                nc.sync.dma_start(
                    xTp[dp][:, j, :],
                    xT_d[(2 * dp + j) * 128:(2 * dp + j + 1) * 128, :])

        # merged o accumulator (fp8, /OSCALE) for the out-projection
        opool = ctx.enter_context(tc.tile_pool(name="oacc", bufs=1))
        o_all = opool.tile([128, NP, SH], fp8, tag="oall", name="oall")

        # merged wo (fp8, xOSCALE), prefetched during the pipeline
        wop = ctx.enter_context(tc.tile_pool(name="wo", bufs=1))
        wo_all = {}

        def load_wo():
            wo_all["t"] = wop.tile([128, NP, D], fp8, tag="woall",
                                   name="woall")
            for p in range(NP):
                nc.sync.dma_start(wo_all["t"][:, p, :], wo_d[p])

        # final-phase tiles that must coexist with the attention pools
        fin = ctx.enter_context(tc.tile_pool(name="fin", bufs=1))
        gamma_bc = fin.tile([128, D], bf16, tag="gamma", name="gamma",
                            bufs=1)
        nc.sync.dma_start(gamma_bc[:], gamma_d[:].partition_broadcast(128))
        beta_bc = fin.tile([128, D], bf16, tag="beta", name="beta", bufs=1)
        nc.sync.dma_start(beta_bc[:], beta_d[:].partition_broadcast(128))
        y_tiles = {}

        # ---- per-head-pair pools (double buffered across hp) -------------
        hp_ctx = ExitStack()
        kqv = hp_ctx.enter_context(tc.tile_pool(name="kqv", bufs=2))
        wts = hp_ctx.enter_context(tc.tile_pool(name="wts", bufs=2))
        vxp = hp_ctx.enter_context(tc.tile_pool(name="vxp", bufs=2))
        ppsum = hp_ctx.enter_context(
            tc.tile_pool(name="ppsum", bufs=2, space=PS))
        scp = hp_ctx.enter_context(tc.tile_pool(name="scp", bufs=2, space=PS))
        ovp = hp_ctx.enter_context(tc.tile_pool(name="ovp", bufs=1, space=PS))
        exp_p = hp_ctx.enter_context(tc.tile_pool(name="exp", bufs=3))
        nop = hp_ctx.enter_context(tc.tile_pool(name="norm", bufs=1))

        exp_ctr = [0]

        def emit_exp(dst_fp8, src_psum):
            """One [128, 1024] exp: ScalarE exact or DVE Schraudolph."""
            i = exp_ctr[0]
            exp_ctr[0] += 1
            if (i % 8) in exp_dve_slots:
                nc.vector.tensor_scalar(
                    dst_fp8.bitcast(u8), src_psum, LOG2E, SCHRAU_B,
                    mult_op, add_op)
            else:
                nc.scalar.activation(dst_fp8, src_psum, Exp, scale=0.125)

        evac_ctr = [0]

        def emit_kq_evac(dst, src_psum, bias):
            i = evac_ctr[0]
            evac_ctr[0] += 1
            if kq_evac_scalar_every and i % kq_evac_scalar_every == 0:
                nc.scalar.activation(dst, src_psum, Copy, bias=bias)
            else:
                nc.vector.tensor_scalar(dst, src_psum, bias, None, add_op)

        def load_kq_weights(hp):
            out = {}
            for nm in ("wk_g", "wq_g", "wk_l", "wq_l"):
                t = wts.tile([128, ND * 128], fp8, tag=nm, name=nm)
                for d in range(ND):
                    nc.sync.dma_start(t[:, d * 128:(d + 1) * 128],
                                      w_d[nm][hp, d])
                out[nm] = t
            return out

        def load_v_weights(grp):
            out = {}
            for nm in ("wv_g", "wv_l"):
                t = wts.tile([128, ND, GHP * 128], fp8, tag=f"{nm}4",
                             name=f"{nm}4", bufs=1)
                for d in range(ND):
                    nc.sync.dma_start(t[:, d, :], wv_d[nm][grp, d])
                out[nm] = t
            return out

        def vproj_gen(grp, wv, dst):
            """V projections for head-pair group grp (4 head-pairs).
            dst['vg'/'vl'] maps hp -> grouped tile [128, 2sub, 2j, GHP, 80]
            (all GHP head-pairs share one tile per (key, t-pair))."""
            for nm, n_t, key in (("wv_g", NST, "vg"), ("wv_l", NQT, "vl")):
                tiles = []
                for t in range(n_t):
                    pt = ppsum.tile([128, 512], f32, tag="pp", name="pp")
                    for dp in range(ND // 2):
                        nc.tensor.matmul(
                            pt[:, 0:GHP * 128],
                            xTp[dp][:, :, t * 128:(t + 1) * 128],
                            wv[nm][:].rearrange(
                                "p (dp j) c -> p dp j c", j=2)[:, dp],
                            start=(dp == 0), stop=(dp == ND // 2 - 1),
                            perf_mode=DR)
                    if t % 2 == 0:
                        vt = vxp.tile([128, 2, 2, GHP, 80], fp8,
                                      tag=f"{key}{t // 2}",
                                      name=f"{key}{t // 2}")
                        tiles.append(vt)
                        dst[key][grp * (n_t // 2) + t // 2] = vt
                        nc.vector.memset(vt[:, :, :, :, 64:65], 1.0)
                    vt = tiles[t // 2]
                    # one evac per s-tile: [128, (sub, ghp, 64)] <- pt
                    nc.vector.tensor_copy(
                        vt[:, :, t % 2, :, 0:64],
                        pt[:, 0:GHP * 128].rearrange(
                            "p (i s k) -> p s i k", s=2, k=64))
                    if t % 2 == 1:
                        yield

        def proj_gen(hp, w, dst):
            """kq projections for head-pair hp: out[hkp, s] = (x@w)^T + b."""
            for nm, s_len, key in (("wk_g", S, "kg"), ("wq_g", SH, "qg"),
                                   ("wk_l", SH, "kl"), ("wq_l", SH, "ql")):
                ot = kqv.tile([128, s_len], bf16, tag=key, name=key)
                dst[key] = ot
                bias = bcol_sb["b" + nm[1:]][hp]
                for so, sl in _chunks(s_len, 512):
                    pt = ppsum.tile([128, 512], f32, tag="pp", name="pp")
                    for dp in range(ND // 2):
                        nc.tensor.matmul(
                            pt[:, 0:sl],
                            w[nm][:].rearrange(
                                "p (dp j c) -> p dp j c", j=2, c=128)[:, dp],
                            xTp[dp][:, :, so:so + sl],
                            start=(dp == 0), stop=(dp == ND // 2 - 1),
                            perf_mode=DR)
                    emit_kq_evac(ot[:, so:so + sl], pt[:, 0:sl], bias)
                    yield

        # -ln(r) via Schraudolph log on the BITS of r (DVE, exact-enough);
        # then exp on ScalarE (Exp table already resident -> no table thrash)
        NLOG_A = -float(np.log(2.0) / (1 << 23))
        NLOG_B = float((127.0 - 0.0430) * np.log(2.0))

        def normalize(opair, qo, hp, st, tagix):
            """softmax-normalize the o pair [65, 2, QC] into o_all; rowsum
            reciprocal computed as exp(-ln~ r) with a bit-trick log."""
            lnr = nop.tile([1, 2, QC], f32, tag=f"lr{tagix}",
                           name=f"lr{tagix}")
            nc.vector.tensor_scalar(lnr[:], opair[64:65, :, :].bitcast(u32),
                                    NLOG_A, NLOG_B, mult_op, add_op)
            rinv = nop.tile([1, 2, QC], f32, tag=f"ri{tagix}",
                            name=f"ri{tagix}")
            nc.scalar.activation(rinv[:], lnr[:], Exp,
                                 bias=-float(np.log(OSCALE)))
            p = p_of(st, hp)
            for sub in range(2):
                rb = nop.tile([64, QC], f32, tag=f"rb{sub}{tagix}",
                              name=f"rb{sub}{tagix}")
                nc.gpsimd.partition_broadcast(rb[:], rinv[0:1, sub, :])
                nc.vector.tensor_tensor(
                    o_all[sub * 64:sub * 64 + 64, p, qo:qo + QC],
                    opair[0:64, sub, :], rb[:], mult_op)

        def attn_gen(hp, src):
            """Attention (global + local) for head-pair hp."""
            kg, qg = src["kg"], src["qg"]
            kl, ql_ = src["kl"], src["ql"]
            # ---- global: q-chunks outer, s-tile-pairs inner --------------
            for qc in range(NQC):
                qo = qc * QC
                opair = ovp.tile([65, 2, QC], f32, tag="o", name="o")
                for tp in range(NST // 2):
                    ex = exp_p.tile([128, 2, 2, QC], fp8, tag="ex", name="ex")
                    for j in range(2):
                        t = 2 * tp + j
                        sc = scp.tile([128, 2, QC], f32, tag="sc", name="sc")
                        for sub in range(2):
                            po = sub * 64
                            nc.tensor.matmul(
                                sc[:, sub, :],
                                kg[po:po + 64, t * 128:(t + 1) * 128],
                                qg[po:po + 64, qo:qo + QC],
                                start=True, stop=True)
                        emit_exp(ex[:, j], sc[:])
                    for sub in range(2):
                        vt = vkeys["vg"][(hp // GHP) * (NST // 2) + tp]
                        nc.tensor.matmul(
                            opair[:, sub, :],
                            vt[:, sub, :, hp % GHP, 0:65],
                            ex[:, :, sub, :],
                            start=(tp == 0), stop=(tp == NST // 2 - 1),
                            perf_mode=DR)
                    yield
                normalize(opair, qo, hp, "g", qc)
                yield
            # ---- local: q-chunk = window pair, DoubleRow over ss ---------
            for wp in range(NWH // 2):
                qo = wp * QC
                opair = ovp.tile([65, 2, QC], f32, tag="o", name="o")
                ex = exp_p.tile([128, 2, 2, QC], fp8, tag="ex", name="ex")
                for ss in range(2):
                    sc = scp.tile([128, 2, QC], f32, tag="sc", name="sc")
                    for sub in range(2):
                        po = sub * 64
                        for wi in range(2):
                            w = 2 * wp + wi
                            st_ = 2 * w + ss
                            nc.tensor.matmul(
                                sc[:, sub, wi * 256:wi * 256 + 256],
                                kl[po:po + 64, st_ * 128:(st_ + 1) * 128],
                                ql_[po:po + 64,
                                    qo + wi * 256:qo + wi * 256 + 256],
                                start=(wi == 0), stop=(wi == 1))
                    emit_exp(ex[:, ss], sc[:])
                for sub in range(2):
                    for wi in range(2):
                        w = 2 * wp + wi
                        vt = vkeys["vl"][(hp // GHP) * (NQT // 2) + w]
                        nc.tensor.matmul(
                            opair[:, sub, wi * 256:wi * 256 + 256],
                            vt[:, sub, :, hp % GHP, 0:65],
                            ex[:, :, sub, wi * 256:wi * 256 + 256],
                            start=(wi == 0), stop=(wi == 1),
                            perf_mode=DR)
                yield
                normalize(opair, qo, hp, "l", 2 + wp)
                yield

        def outproj_partial_gen():
            """Out-projection partial sums (pairs 0..NP//2-2) + residual,
            overlapped with the last attention; leaves y = x + bo + partial.
            (bo_eff is folded into xq host-side.)"""
            wo_t = wo_all["t"]
            for qt in range(NQT):
                xq_t = fin.tile([128, D], f32, tag="xq", name="xq", bufs=2)
                nc.sync.dma_start(xq_t[:], xq_d[qt * 128:(qt + 1) * 128, :])
                y = fin.tile([128, D], bf16, tag="y", name="y", bufs=NQT)
                y_tiles[qt] = y
                for do, dl in _chunks(D, 512):
                    pt = ppsum.tile([128, 512], f32, tag="pp", name="pp")
                    for k in range(NP // 2 - 1):
                        nc.tensor.matmul(
                            pt[:, 0:dl],
                            o_all[:, 2 * k:2 * k + 2,
                                  qt * 128:(qt + 1) * 128],
                            wo_t[:, 2 * k:2 * k + 2, do:do + dl],
                            start=(k == 0), stop=(k == NP // 2 - 2),
                            perf_mode=DR)
                    nc.vector.tensor_tensor(
                        y[:, do:do + dl], pt[:, 0:dl],
                        xq_t[:, do:do + dl], add_op)
                    yield

        # ---- software-pipelined main loop over head-pairs ----------------
        import itertools
        kq_w = load_kq_weights(0)
        v_w = load_v_weights(0)
        src = {}
        prev_src = None
        vkeys = {"vg": {}, "vl": {}}
        for hp in range(NHK):
            if hp == 0:
                pg = itertools.chain(proj_gen(hp, kq_w, src),
                                     vproj_gen(0, v_w, vkeys))
            elif hp % GHP == 0:
                pg = itertools.chain(vproj_gen(hp // GHP, v_w, vkeys),
                                     proj_gen(hp, kq_w, src))
            else:
                pg = proj_gen(hp, kq_w, src)
            ag = attn_gen(hp - 1, prev_src) if prev_src is not None else None
            if hp == 2:
                load_wo()
            if hp + 1 < NHK:
                kq_next = load_kq_weights(hp + 1)
            if hp % GHP == 1 and hp + GHP < NHK + 1:
                v_next = load_v_weights((hp + GHP) // GHP)
            # interleave: 2 attention steps per projection step
            done_p, done_a = False, ag is None
            while not (done_p and done_a):
                if not done_a:
                    done_a = next(ag, "END") == "END"
                if not done_p:
                    done_p = next(pg, "END") == "END"
                if not done_a:
                    done_a = next(ag, "END") == "END"
            if hp + 1 < NHK:
                kq_w = kq_next
            if hp % GHP == 1 and hp + GHP < NHK + 1:
                v_w = v_next
            prev_src, src = src, {}
        ag = attn_gen(NHK - 1, prev_src)
        pg = outproj_partial_gen()
        done_p = done_a = False
        while not (done_p and done_a):
            if not done_a:
                done_a = next(ag, "END") == "END"
            if not done_p:
                done_p = next(pg, "END") == "END"
            if not done_a:
                done_a = next(ag, "END") == "END"
        hp_ctx.close()

        # ---- finish: last pair (g7, l7), then layernorm ------------------
        wo_t = wo_all["t"]
        with tc.tile_pool(name="ypsum", bufs=2, space=PS) as ypp, \
             tc.tile_pool(name="ln", bufs=2) as lnp:
            for qt in range(NQT):
                y = y_tiles[qt]
                for do, dl in _chunks(D, 512):
                    ps_y = ypp.tile([128, 512], f32, tag="py", name="py")
                    nc.tensor.matmul(
                        ps_y[:, 0:dl],
                        o_all[:, NP - 2:NP, qt * 128:(qt + 1) * 128],
                        wo_t[:, NP - 2:NP, do:do + dl],
                        start=True, stop=True, perf_mode=DR)
                    nc.vector.tensor_tensor(y[:, do:do + dl], y[:, do:do + dl],
                                            ps_y[:, 0:dl], add_op)
                # layernorm: bn_stats/aggr for mean+var in two passes
                st6 = lnp.tile([128, 2, 6], f32, tag="st6", name="st6")
                nc.vector.bn_stats(st6[:, 0, :], y[:, 0:512])
                nc.vector.bn_stats(st6[:, 1, :], y[:, 512:1024])
                mv = lnp.tile([128, 2], f32, tag="mv", name="mv")
                nc.vector.bn_aggr(mv[:], st6[:])
                sd = lnp.tile([128, 1], f32, tag="sd", name="sd")
                nc.scalar.activation(sd[:], mv[:, 1:2], Sqrt,
                                     bias=eps_col[:])
                rstd = lnp.tile([128, 1], f32, tag="rstd", name="rstd")
                nc.vector.reciprocal(rstd[:], sd[:])
                bco = lnp.tile([128, 1], f32, tag="bco", name="bco")
                nc.vector.tensor_tensor(bco[:], mv[:, 0:1], rstd[:], mult_op)
                nc.vector.tensor_scalar_mul(bco[:], bco[:], -1.0)
                t1 = lnp.tile([128, D], bf16, tag="t1", name="t1")
                nc.vector.tensor_scalar(t1[:], y[:], rstd[:], bco[:],
                                        mult_op, add_op)
                t2 = lnp.tile([128, D], bf16, tag="t2", name="t2")
                nc.vector.tensor_tensor(t2[:], t1[:], gamma_bc[:], mult_op)
                ot = lnp.tile([128, D], bf16, tag="ot", name="ot")
                nc.vector.tensor_tensor(ot[:], t2[:], beta_bc[:], add_op)
                nc.sync.dma_start(out_d[qt * 128:(qt + 1) * 128, :], ot[:])

    nc.compile()
    return nc


def make_in_maps(inputs, cfg=None):
    """Build per-core input maps from the full (unsharded) problem inputs."""
    cfg = dict(cfg or FULL_CFG)
    S, D, H, K = cfg["S"], cfg["D"], cfg["H"], cfg["K"]
    HK = H * K
    SH = S // 2
    NHK = HK // 128
    ND = D // 128
    NGRP = 2
    GW = HK // NGRP
    NP = 2 * NHK

    def np32(a):
        return np.asarray(a, dtype=np.float32)

    shared = {}
    for nm, key in (("wq_g", "gWq"), ("wk_g", "gWk"),
                    ("wq_l", "lWq"), ("wk_l", "lWk")):
        w = np32(inputs[key]).reshape(D, HK)
        shared[nm] = np.ascontiguousarray(
            w.reshape(ND, 128, NHK, 128).transpose(2, 0, 1, 3)).astype(FP8)
    for nm, key in (("wv_g", "gWv"), ("wv_l", "lWv")):
        w = np32(inputs[key]).reshape(D, HK)
        shared[nm] = np.ascontiguousarray(
            w.reshape(ND, 128, NGRP, GW).transpose(2, 0, 1, 3)).astype(FP8)
    # merged wo in p_of order: g0..g6, l0..l6, g7, l7; scaled by OSCALE
    wo_g = np32(inputs["gWo"]).reshape(HK, D)
    wo_l = np32(inputs["lWo"]).reshape(HK, D)
    wo = np.empty((NP, 128, D), np.float32)
    for hp in range(NHK):
        pg = hp if hp < NHK - 1 else NP - 2
        pl = (NHK - 1) + hp if hp < NHK - 1 else NP - 1
        wo[pg] = wo_g[hp * 128:(hp + 1) * 128]
        wo[pl] = wo_l[hp * 128:(hp + 1) * 128]
    shared["wo"] = (wo * OSCALE).astype(FP8)
    for nm, key in (("bq_g", "gbq"), ("bk_g", "gbk"),
                    ("bq_l", "lbq"), ("bk_l", "lbk")):
        shared[nm] = np.ascontiguousarray(np32(inputs[key]).reshape(NHK, 128))
    # bv folds into the out-projection bias; that + bo fold into xq
    bo_eff = (np32(inputs["gbo"]) + np32(inputs["lbo"]) +
              np32(inputs["gbv"]).reshape(HK) @ wo_g +
              np32(inputs["lbv"]).reshape(HK) @ wo_l)
    shared["gamma"] = np32(inputs["gamma"]).reshape(1, D).astype(BF16)
    shared["beta"] = np32(inputs["beta"]).reshape(1, D).astype(BF16)

    x = np32(inputs["x"])
    in_maps = []
    for c in range(N_CORES):
        b, half = divmod(c, 2)
        xb = x[b]
        # own half first (queries/local), other half second; global attention
        # is invariant to key/value column order
        xperm = np.concatenate([xb[half * SH:(half + 1) * SH],
                                xb[(1 - half) * SH:(2 - half) * SH]], axis=0)
        m = dict(shared)
        m["xT"] = np.ascontiguousarray(xperm.T).astype(FP8)
        m["xq"] = np.ascontiguousarray(xperm[0:SH] + bo_eff[None, :])
        in_maps.append(m)
    return in_maps


def assemble_out(results, cfg=None):
    cfg = dict(cfg or FULL_CFG)
    S, D = cfg["S"], cfg["D"]
    SH = S // 2
    B = N_CORES // 2
    out = np.empty((B, S, D), np.float32)
    for c in range(N_CORES):
        b, half = divmod(c, 2)
        out[b, half * SH:(half + 1) * SH] = np.asarray(
            results[c]["out"]).astype(np.float32)
    return out


_NC_CACHE = {}


def kernel(**inputs):
    from concourse.bass_utils import run_bass_kernel_spmd
    if "nc" not in _NC_CACHE:
        _NC_CACHE["nc"] = build_nc()
    nc = _NC_CACHE["nc"]
    in_maps = make_in_maps(inputs)
    res = run_bass_kernel_spmd(nc, in_maps, list(range(N_CORES)))
    return assemble_out(res.results)


# revision 14
# speedup vs baseline: 1.1767x; 1.0687x over previous
"""Trainium2 Bass kernel for LocalGlobalSelfAttention (v4).

Sharding: 8 cores = 4 batches x 2 sequence-halves (no collectives).
Each core computes, for its (batch b, half h):
  - global attention: queries = its half (SH rows), keys/values = full seq
  - local windowed attention: fully contained in its half
  - output projections (g+l accumulated in PSUM) + residual + layernorm

v4 changes over v3 (566us baseline):
  - exp split across ScalarE (exact, fp8 out) and DVE (Schraudolph: one
    tensor_scalar affine + f32->uint8 convert whose bits, read as fp8e4m3,
    approximate 2^x; softmax renormalization cancels the systematic bias)
  - softmax reciprocal on ScalarE as exp(-ln r) (same activation table set
    as Exp -> no table thrash); paired [65,2,QC] PSUM o tiles so rowsum
    extraction is one op per (hp, chunk)
  - bv folded into the output-projection bias, and that + bo folded into
    the residual input xq host-side; V tiles grouped per head-pair-group so
    the PSUM->SBUF evac is one copy per s-tile
  - out-projection in fp8 DoubleRow over merged o/wo tiles (pair order
    g0..g6,l0..l6,g7,l7 so the last pair is exactly what the final phase
    adds); wo scaled x4 host-side, o scaled /4 via the reciprocal bias to
    dodge fp8 subnormals
  - layernorm via bn_stats + fused tensor_scalar, gamma/beta on GpSimd,
    bf16 output
"""

import numpy as np
import ml_dtypes
from contextlib import ExitStack

BF16 = ml_dtypes.bfloat16
FP8 = ml_dtypes.float8_e4m3

FULL_CFG = dict(S=2048, D=1024, H=16, K=64, NW=8)
N_CORES = 8
LN_EPS = 1e-3

LOG2E = 1.4426950408889634
SCHRAU_B = 55.65          # exp bias for round-to-nearest f32->uint8
OSCALE = 1.0              # optional wo x / o ÷ scale split (fp8 headroom)
# exp instances assigned to DVE when (counter % 16) in this set
EXP_DVE_SLOTS = (3, 8, 13)


def _chunks(total, size):
    return [(o, min(size, total - o)) for o in range(0, total, size)]


def build_nc(cfg=None, n_dev=N_CORES, exp_dve_slots=EXP_DVE_SLOTS,
             kq_evac_scalar_every=0):
    """Build + compile the per-core Bass program (SPMD, same on all cores)."""
    import concourse.bass as bass
    import concourse.tile as tile
    import concourse.mybir as mybir
    from concourse import bacc

    cfg = dict(cfg or FULL_CFG)
    S, D, H, K, NW = cfg["S"], cfg["D"], cfg["H"], cfg["K"], cfg["NW"]
    HK = H * K
    SH = S // 2          # per-core query rows (half the sequence)
    WIN = S // NW        # local attention window
    NWH = SH // WIN      # windows in this core's half
    assert K == 64 and D % 128 == 0 and HK % 128 == 0

    ND = D // 128        # d-tiles
    NHK = HK // 128      # head-pair tiles (2 heads each)
    NST = S // 128       # s-tiles (full seq)
    NQT = SH // 128      # q-tiles (half seq)
    QC = 512             # query chunk for AV / o accumulation
    NQC = SH // QC
    NGRP = 2             # v-projection groups (4 head-pairs each)
    GHP = NHK // NGRP
    NP = 2 * NHK         # merged out-projection tiles (g and l)

    f32 = mybir.dt.float32
    bf16 = mybir.dt.bfloat16
    fp8 = mybir.dt.float8e4
    u8 = mybir.dt.uint8
    u32 = mybir.dt.uint32
    DR = mybir.MatmulPerfMode.DoubleRow
    Exp = mybir.ActivationFunctionType.Exp
    Copy = mybir.ActivationFunctionType.Copy
    Sqrt = mybir.ActivationFunctionType.Sqrt
    add_op = mybir.AluOpType.add
    mult_op = mybir.AluOpType.mult
    sub_op = mybir.AluOpType.subtract

    # out-projection tile order: g0..g6, l0..l6, g7, l7
    def p_of(st, hp):
        if hp < NHK - 1:
            return hp if st == "g" else (NHK - 1) + hp
        return NP - 2 if st == "g" else NP - 1

    nc = bacc.Bacc("TRN2", target_bir_lowering=False, debug=False,
                   num_devices=n_dev)

    # ---- DRAM parameters -------------------------------------------------
    xT_d = nc.dram_tensor("xT", [D, S], fp8, kind="ExternalInput")
    xq_d = nc.dram_tensor("xq", [SH, D], f32, kind="ExternalInput")
    w_d = {}
    for nm in ("wq_g", "wk_g", "wq_l", "wk_l"):
        w_d[nm] = nc.dram_tensor(nm, [NHK, ND, 128, 128], fp8,
                                 kind="ExternalInput")
    wv_d = {}
    for nm in ("wv_g", "wv_l"):
        wv_d[nm] = nc.dram_tensor(nm, [NGRP, ND, 128, GHP * 128], fp8,
                                  kind="ExternalInput")
    # merged wo, pre-ordered host-side to the p_of layout, scaled x4, fp8
    wo_d = nc.dram_tensor("wo", [NP, 128, D], fp8, kind="ExternalInput")
    bcol_d = {}
    for nm in ("bq_g", "bk_g", "bq_l", "bk_l"):
        bcol_d[nm] = nc.dram_tensor(nm, [NHK, 128], f32, kind="ExternalInput")
    gamma_d = nc.dram_tensor("gamma", [1, D], bf16, kind="ExternalInput")
    beta_d = nc.dram_tensor("beta", [1, D], bf16, kind="ExternalInput")
    out_d = nc.dram_tensor("out", [SH, D], bf16, kind="ExternalOutput")

    PS = bass.MemorySpace.PSUM

    with tile.TileContext(nc) as tc, ExitStack() as ctx:
        # ---- constants (live whole kernel) -------------------------------
        cpool = ctx.enter_context(tc.tile_pool(name="consts", bufs=1))
        eps_col = cpool.tile([128, 1], f32, tag="eps", name="eps")
        nc.vector.memset(eps_col[:], float(LN_EPS))
        bcol_sb = {}
        for nm, d in bcol_d.items():
            cols = []
            for j in range(NHK):
                t = cpool.tile([128, 1], f32, tag=f"{nm}{j}", name=f"{nm}{j}")
                nc.sync.dma_start(t[:], d[j, :].rearrange("(a b) -> a b", b=1))
                cols.append(t)
            bcol_sb[nm] = cols

        # x^T resident in SBUF for all projections
        xpool = ctx.enter_context(tc.tile_pool(name="xin", bufs=1))
        xTp = [xpool.tile([128, 2, S], fp8, tag=f"xt{dp}", name=f"xt{dp}")
               for dp in range(ND // 2)]
        for dp in range(ND // 2):
            for j in range(2):
                nc.sync.dma_start(
                    xTp[dp][:, j, :],
                    xT_d[(2 * dp + j) * 128:(2 * dp + j + 1) * 128, :])

        # merged o accumulator (fp8, /OSCALE) for the out-projection
        opool = ctx.enter_context(tc.tile_pool(name="oacc", bufs=1))
        o_all = opool.tile([128, NP, SH], fp8, tag="oall", name="oall")

        # merged wo (fp8, xOSCALE), prefetched during the pipeline
        wop = ctx.enter_context(tc.tile_pool(name="wo", bufs=1))
        wo_all = {}

        def load_wo():
            wo_all["t"] = wop.tile([128, NP, D], fp8, tag="woall",
                                   name="woall")
            for p in range(NP):
                nc.sync.dma_start(wo_all["t"][:, p, :], wo_d[p])

        # final-phase tiles that must coexist with the attention pools
        fin = ctx.enter_context(tc.tile_pool(name="fin", bufs=1))
        gamma_bc = fin.tile([128, D], bf16, tag="gamma", name="gamma",
                            bufs=1)
        nc.sync.dma_start(gamma_bc[:], gamma_d[:].partition_broadcast(128))
        beta_bc = fin.tile([128, D], bf16, tag="beta", name="beta", bufs=1)
        nc.sync.dma_start(beta_bc[:], beta_d[:].partition_broadcast(128))
        y_tiles = {}

        # ---- per-head-pair pools (double buffered across hp) -------------
        hp_ctx = ExitStack()
        kqv = hp_ctx.enter_context(tc.tile_pool(name="kqv", bufs=2))
        wts = hp_ctx.enter_context(tc.tile_pool(name="wts", bufs=2))
        vxp = hp_ctx.enter_context(tc.tile_pool(name="vxp", bufs=2))
        ppsum = hp_ctx.enter_context(
            tc.tile_pool(name="ppsum", bufs=2, space=PS))
        scp = hp_ctx.enter_context(tc.tile_pool(name="scp", bufs=2, space=PS))
        ovp = hp_ctx.enter_context(tc.tile_pool(name="ovp", bufs=1, space=PS))
        exp_p = hp_ctx.enter_context(tc.tile_pool(name="exp", bufs=3))
        nop = hp_ctx.enter_context(tc.tile_pool(name="norm", bufs=1))

        exp_ctr = [0]

        def emit_exp(dst_fp8, src_psum):
            """One [128, 1024] exp: ScalarE exact or DVE Schraudolph."""
            i = exp_ctr[0]
            exp_ctr[0] += 1
            if (i % 16) in exp_dve_slots:
                nc.vector.tensor_scalar(
                    dst_fp8.bitcast(u8), src_psum, LOG2E, SCHRAU_B,
                    mult_op, add_op)
            else:
                nc.scalar.activation(dst_fp8, src_psum, Exp, scale=0.125)

        evac_ctr = [0]

        def emit_kq_evac(dst, src_psum, bias):
            i = evac_ctr[0]
            evac_ctr[0] += 1
            if kq_evac_scalar_every and i % kq_evac_scalar_every == 0:
                nc.scalar.activation(dst, src_psum, Copy, bias=bias)
            else:
                nc.vector.tensor_scalar(dst, src_psum, bias, None, add_op)

        def load_kq_weights(hp):
            out = {}
            for nm in ("wk_g", "wq_g", "wk_l", "wq_l"):
                t = wts.tile([128, ND * 128], fp8, tag=nm, name=nm)
                for d in range(ND):
                    nc.sync.dma_start(t[:, d * 128:(d + 1) * 128],
                                      w_d[nm][hp, d])
                out[nm] = t
            return out

        def load_v_weights(grp):
            out = {}
            for nm in ("wv_g", "wv_l"):
                t = wts.tile([128, ND, GHP * 128], fp8, tag=f"{nm}4",
                             name=f"{nm}4", bufs=1)
                for d in range(ND):
                    nc.sync.dma_start(t[:, d, :], wv_d[nm][grp, d])
                out[nm] = t
            return out

        def vproj_gen(grp, wv, dst):
            """V projections for head-pair group grp (4 head-pairs).
            dst['vg'/'vl'] maps hp -> grouped tile [128, 2sub, 2j, GHP, 80]
            (all GHP head-pairs share one tile per (key, t-pair))."""
            for nm, n_t, key in (("wv_g", NST, "vg"), ("wv_l", NQT, "vl")):
                tiles = []
                for t in range(n_t):
                    pt = ppsum.tile([128, 512], f32, tag="pp", name="pp")
                    for dp in range(ND // 2):
                        nc.tensor.matmul(
                            pt[:, 0:GHP * 128],
                            xTp[dp][:, :, t * 128:(t + 1) * 128],
                            wv[nm][:].rearrange(
                                "p (dp j) c -> p dp j c", j=2)[:, dp],
                            start=(dp == 0), stop=(dp == ND // 2 - 1),
                            perf_mode=DR)
                    if t % 2 == 0:
                        vt = vxp.tile([128, 2, 2, GHP, 80], fp8,
                                      tag=f"{key}{t // 2}",
                                      name=f"{key}{t // 2}")
                        tiles.append(vt)
                        dst[key][grp * (n_t // 2) + t // 2] = vt
                        nc.vector.memset(vt[:, :, :, :, 64:65], 1.0)
                    vt = tiles[t // 2]
                    # one evac per s-tile: [128, (sub, ghp, 64)] <- pt
                    nc.vector.tensor_copy(
                        vt[:, :, t % 2, :, 0:64],
                        pt[:, 0:GHP * 128].rearrange(
                            "p (i s k) -> p s i k", s=2, k=64))
                    if t % 2 == 1:
                        yield

        def proj_gen(hp, w, dst):
            """kq projections for head-pair hp: out[hkp, s] = (x@w)^T + b."""
            for nm, s_len, key in (("wk_g", S, "kg"), ("wq_g", SH, "qg"),
                                   ("wk_l", SH, "kl"), ("wq_l", SH, "ql")):
                ot = kqv.tile([128, s_len], bf16, tag=key, name=key)
                dst[key] = ot
                bias = bcol_sb["b" + nm[1:]][hp]
                for so, sl in _chunks(s_len, 512):
                    pt = ppsum.tile([128, 512], f32, tag="pp", name="pp")
                    for dp in range(ND // 2):
                        nc.tensor.matmul(
                            pt[:, 0:sl],
                            w[nm][:].rearrange(
                                "p (dp j c) -> p dp j c", j=2, c=128)[:, dp],
                            xTp[dp][:, :, so:so + sl],
                            start=(dp == 0), stop=(dp == ND // 2 - 1),
                            perf_mode=DR)
                    emit_kq_evac(ot[:, so:so + sl], pt[:, 0:sl], bias)
                    yield

        # 1/r in ONE DVE op: bit-trick reciprocal seed bitcast(K - bits(r)).
        # ~5% gain ripple on the softmax denominator; diluted ~100x by the
        # residual stream. K calibrated for r in [80, 8000]; a /OSCALE is a
        # -log2(OSCALE)*2^23 shift of K.
        RECIP_K = float((254.0 - 0.100 - np.log2(OSCALE)) * (1 << 23))

        def normalize(opair, qo, hp, st, tagix):
            """softmax-normalize the o pair [65, 2, QC] into o_all."""
            rinv = nop.tile([1, 2, QC], f32, tag=f"ri{tagix}",
                            name=f"ri{tagix}")
            nc.vector.tensor_scalar(rinv[:].bitcast(u32),
                                    opair[64:65, :, :].bitcast(u32),
                                    -1.0, RECIP_K, mult_op, add_op)
            p = p_of(st, hp)
            for sub in range(2):
                rb = nop.tile([64, QC], f32, tag=f"rb{sub}{tagix}",
                              name=f"rb{sub}{tagix}")
                nc.gpsimd.partition_broadcast(rb[:], rinv[0:1, sub, :])
                nc.vector.tensor_tensor(
                    o_all[sub * 64:sub * 64 + 64, p, qo:qo + QC],
                    opair[0:64, sub, :], rb[:], mult_op)

        def attn_gen(hp, src):
            """Attention (global + local) for head-pair hp."""
            kg, qg = src["kg"], src["qg"]
            kl, ql_ = src["kl"], src["ql"]
            # ---- global: q-chunks outer, s-tile-pairs inner --------------
            for qc in range(NQC):
                qo = qc * QC
                opair = ovp.tile([65, 2, QC], f32, tag="o", name="o")
                for tp in range(NST // 2):
                    ex = exp_p.tile([128, 2, 2, QC], fp8, tag="ex", name="ex")
                    for j in range(2):
                        t = 2 * tp + j
                        sc = scp.tile([128, 2, QC], f32, tag="sc", name="sc")
                        for sub in range(2):
                            po = sub * 64
                            nc.tensor.matmul(
                                sc[:, sub, :],
                                kg[po:po + 64, t * 128:(t + 1) * 128],
                                qg[po:po + 64, qo:qo + QC],
                                start=True, stop=True)
                        emit_exp(ex[:, j], sc[:])
                    for sub in range(2):
                        vt = vkeys["vg"][(hp // GHP) * (NST // 2) + tp]
                        nc.tensor.matmul(
                            opair[:, sub, :],
                            vt[:, sub, :, hp % GHP, 0:65],
                            ex[:, :, sub, :],
                            start=(tp == 0), stop=(tp == NST // 2 - 1),
                            perf_mode=DR)
                    yield
                normalize(opair, qo, hp, "g", qc)
                yield
            # ---- local: q-chunk = window pair, DoubleRow over ss ---------
            for wp in range(NWH // 2):
                qo = wp * QC
                opair = ovp.tile([65, 2, QC], f32, tag="o", name="o")
                ex = exp_p.tile([128, 2, 2, QC], fp8, tag="ex", name="ex")
                for ss in range(2):
                    sc = scp.tile([128, 2, QC], f32, tag="sc", name="sc")
                    for sub in range(2):
                        po = sub * 64
                        for wi in range(2):
                            w = 2 * wp + wi
                            st_ = 2 * w + ss
                            nc.tensor.matmul(
                                sc[:, sub, wi * 256:wi * 256 + 256],
                                kl[po:po + 64, st_ * 128:(st_ + 1) * 128],
                                ql_[po:po + 64,
                                    qo + wi * 256:qo + wi * 256 + 256],
                                start=(wi == 0), stop=(wi == 1))
                    emit_exp(ex[:, ss], sc[:])
                for sub in range(2):
                    for wi in range(2):
                        w = 2 * wp + wi
                        vt = vkeys["vl"][(hp // GHP) * (NQT // 2) + w]
                        nc.tensor.matmul(
                            opair[:, sub, wi * 256:wi * 256 + 256],
                            vt[:, sub, :, hp % GHP, 0:65],
                            ex[:, :, sub, wi * 256:wi * 256 + 256],
                            start=(wi == 0), stop=(wi == 1),
                            perf_mode=DR)
                yield
                normalize(opair, qo, hp, "l", 2 + wp)
                yield

        def outproj_partial_gen():
            """Out-projection partial sums (pairs 0..NP//2-2) + residual,
            overlapped with the last attention; leaves y = x + bo + partial.
            (bo_eff is folded into xq host-side.)"""
            wo_t = wo_all["t"]
            for qt in range(NQT):
                xq_t = fin.tile([128, D], f32, tag="xq", name="xq", bufs=2)
                nc.sync.dma_start(xq_t[:], xq_d[qt * 128:(qt + 1) * 128, :])
                y = fin.tile([128, D], bf16, tag="y", name="y", bufs=NQT)
                y_tiles[qt] = y
                for do, dl in _chunks(D, 512):
                    pt = ppsum.tile([128, 512], f32, tag="pp", name="pp")
                    for k in range(NP // 2 - 1):
                        nc.tensor.matmul(
                            pt[:, 0:dl],
                            o_all[:, 2 * k:2 * k + 2,
                                  qt * 128:(qt + 1) * 128],
                            wo_t[:, 2 * k:2 * k + 2, do:do + dl],
                            start=(k == 0), stop=(k == NP // 2 - 2),
                            perf_mode=DR)
                    nc.vector.tensor_tensor(
                        y[:, do:do + dl], pt[:, 0:dl],
                        xq_t[:, do:do + dl], add_op)
                    yield

        # ---- software-pipelined main loop over head-pairs ----------------
        import itertools
        kq_w = load_kq_weights(0)
        v_w = load_v_weights(0)
        src = {}
        prev_src = None
        vkeys = {"vg": {}, "vl": {}}
        for hp in range(NHK):
            if hp == 0:
                pg = itertools.chain(proj_gen(hp, kq_w, src),
                                     vproj_gen(0, v_w, vkeys))
            elif hp % GHP == 0:
                pg = itertools.chain(vproj_gen(hp // GHP, v_w, vkeys),
                                     proj_gen(hp, kq_w, src))
            else:
                pg = proj_gen(hp, kq_w, src)
            ag = attn_gen(hp - 1, prev_src) if prev_src is not None else None
            if hp == 2:
                load_wo()
            if hp + 1 < NHK:
                kq_next = load_kq_weights(hp + 1)
            if hp % GHP == 1 and hp + GHP < NHK + 1:
                v_next = load_v_weights((hp + GHP) // GHP)
            # interleave: 2 attention steps per projection step
            done_p, done_a = False, ag is None
            while not (done_p and done_a):
                if not done_a:
                    done_a = next(ag, "END") == "END"
                if not done_p:
                    done_p = next(pg, "END") == "END"
                if not done_a:
                    done_a = next(ag, "END") == "END"
            if hp + 1 < NHK:
                kq_w = kq_next
            if hp % GHP == 1 and hp + GHP < NHK + 1:
                v_w = v_next
            prev_src, src = src, {}
        ag = attn_gen(NHK - 1, prev_src)
        pg = outproj_partial_gen()
        done_p = done_a = False
        while not (done_p and done_a):
            if not done_a:
                done_a = next(ag, "END") == "END"
            if not done_p:
                done_p = next(pg, "END") == "END"
            if not done_a:
                done_a = next(ag, "END") == "END"
        hp_ctx.close()

        # ---- finish: last pair (g7, l7), then layernorm ------------------
        wo_t = wo_all["t"]
        with tc.tile_pool(name="ypsum", bufs=2, space=PS) as ypp, \
             tc.tile_pool(name="ln", bufs=2) as lnp:
            for qt in range(NQT):
                y = y_tiles[qt]
                for do, dl in _chunks(D, 512):
                    ps_y = ypp.tile([128, 512], f32, tag="py", name="py")
                    nc.tensor.matmul(
                        ps_y[:, 0:dl],
                        o_all[:, NP - 2:NP, qt * 128:(qt + 1) * 128],
                        wo_t[:, NP - 2:NP, do:do + dl],
                        start=True, stop=True, perf_mode=DR)
                    nc.vector.tensor_tensor(y[:, do:do + dl], y[:, do:do + dl],
                                            ps_y[:, 0:dl], add_op)
                # layernorm: bn_stats/aggr for mean+var in two passes
                st6 = lnp.tile([128, 2, 6], f32, tag="st6", name="st6")
                nc.vector.bn_stats(st6[:, 0, :], y[:, 0:512])
                nc.vector.bn_stats(st6[:, 1, :], y[:, 512:1024])
                mv = lnp.tile([128, 2], f32, tag="mv", name="mv")
                nc.vector.bn_aggr(mv[:], st6[:])
                sd = lnp.tile([128, 1], f32, tag="sd", name="sd")
                nc.scalar.activation(sd[:], mv[:, 1:2], Sqrt,
                                     bias=eps_col[:])
                rstd = lnp.tile([128, 1], f32, tag="rstd", name="rstd")
                nc.vector.reciprocal(rstd[:], sd[:])
                bco = lnp.tile([128, 1], f32, tag="bco", name="bco")
                nc.vector.tensor_tensor(bco[:], mv[:, 0:1], rstd[:], mult_op)
                nc.vector.tensor_scalar_mul(bco[:], bco[:], -1.0)
                t1 = lnp.tile([128, D], bf16, tag="t1", name="t1")
                nc.vector.tensor_scalar(t1[:], y[:], rstd[:], bco[:],
                                        mult_op, add_op)
                t2 = lnp.tile([128, D], bf16, tag="t2", name="t2")
                nc.vector.tensor_tensor(t2[:], t1[:], gamma_bc[:], mult_op)
                ot = lnp.tile([128, D], bf16, tag="ot", name="ot")
                nc.vector.tensor_tensor(ot[:], t2[:], beta_bc[:], add_op)
                nc.sync.dma_start(out_d[qt * 128:(qt + 1) * 128, :], ot[:])

    nc.compile()
    return nc


def make_in_maps(inputs, cfg=None):
    """Build per-core input maps from the full (unsharded) problem inputs."""
    cfg = dict(cfg or FULL_CFG)
    S, D, H, K = cfg["S"], cfg["D"], cfg["H"], cfg["K"]
    HK = H * K
    SH = S // 2
    NHK = HK // 128
    ND = D // 128
    NGRP = 2
    GW = HK // NGRP
    NP = 2 * NHK

    def np32(a):
        return np.asarray(a, dtype=np.float32)

    shared = {}
    for nm, key in (("wq_g", "gWq"), ("wk_g", "gWk"),
                    ("wq_l", "lWq"), ("wk_l", "lWk")):
        w = np32(inputs[key]).reshape(D, HK)
        shared[nm] = np.ascontiguousarray(
            w.reshape(ND, 128, NHK, 128).transpose(2, 0, 1, 3)).astype(FP8)
    for nm, key in (("wv_g", "gWv"), ("wv_l", "lWv")):
        w = np32(inputs[key]).reshape(D, HK)
        shared[nm] = np.ascontiguousarray(
            w.reshape(ND, 128, NGRP, GW).transpose(2, 0, 1, 3)).astype(FP8)
    # merged wo in p_of order: g0..g6, l0..l6, g7, l7; scaled by OSCALE
    wo_g = np32(inputs["gWo"]).reshape(HK, D)
    wo_l = np32(inputs["lWo"]).reshape(HK, D)
    wo = np.empty((NP, 128, D), np.float32)
    for hp in range(NHK):
        pg = hp if hp < NHK - 1 else NP - 2
        pl = (NHK - 1) + hp if hp < NHK - 1 else NP - 1
        wo[pg] = wo_g[hp * 128:(hp + 1) * 128]
        wo[pl] = wo_l[hp * 128:(hp + 1) * 128]
    shared["wo"] = (wo * OSCALE).astype(FP8)
    for nm, key in (("bq_g", "gbq"), ("bk_g", "gbk"),
                    ("bq_l", "lbq"), ("bk_l", "lbk")):
        shared[nm] = np.ascontiguousarray(np32(inputs[key]).reshape(NHK, 128))
    # bv folds into the out-projection bias; that + bo fold into xq
    bo_eff = (np32(inputs["gbo"]) + np32(inputs["lbo"]) +
              np32(inputs["gbv"]).reshape(HK) @ wo_g +
              np32(inputs["lbv"]).reshape(HK) @ wo_l)
    shared["gamma"] = np32(inputs["gamma"]).reshape(1, D).astype(BF16)
    shared["beta"] = np32(inputs["beta"]).reshape(1, D).astype(BF16)

    x = np32(inputs["x"])
    in_maps = []
    for c in range(N_CORES):
        b, half = divmod(c, 2)
        xb = x[b]
        # own half first (queries/local), other half second; global attention
        # is invariant to key/value column order
        xperm = np.concatenate([xb[half * SH:(half + 1) * SH],
                                xb[(1 - half) * SH:(2 - half) * SH]], axis=0)
        m = dict(shared)
        m["xT"] = np.ascontiguousarray(xperm.T).astype(FP8)
        m["xq"] = np.ascontiguousarray(xperm[0:SH] + bo_eff[None, :])
        in_maps.append(m)
    return in_maps


def assemble_out(results, cfg=None):
    cfg = dict(cfg or FULL_CFG)
    S, D = cfg["S"], cfg["D"]
    SH = S // 2
    B = N_CORES // 2
    out = np.empty((B, S, D), np.float32)
    for c in range(N_CORES):
        b, half = divmod(c, 2)
        out[b, half * SH:(half + 1) * SH] = np.asarray(
            results[c]["out"]).astype(np.float32)
    return out


_NC_CACHE = {}


def kernel(**inputs):
    from concourse.bass_utils import run_bass_kernel_spmd
    if "nc" not in _NC_CACHE:
        _NC_CACHE["nc"] = build_nc()
    nc = _NC_CACHE["nc"]
    in_maps = make_in_maps(inputs)
    res = run_bass_kernel_spmd(nc, in_maps, list(range(N_CORES)))
    return assemble_out(res.results)
